# revision 18
# baseline (speedup 1.0000x reference)
"""HMM forward (negative log-marginal) on 8 TRN2 NeuronCores.

Algorithm: the log-space recurrence
    alpha_t[b,j] = obs_t[b,j] + LSE_i(alpha_{t-1}[b,i] + T_log[j,i])
is run in linear space with per-step host-precomputed normalizers:
    aD_t[j,b] = (eobs_t[j,b] / (sigma_tb * SW)) * sum_i Wq[i,j] * aD_{t-1}[i,b]
where sigma_tb = pi_star . eobs_t[:,b] is a rank-1 prediction of the
per-step growth that keeps aD ~ O(1), and the log scales are summed on
the host:
    -log p = -(log sum_j aD_255 + sum_t log sigma_tb + logC0 - 255*SHIFT).

Performance structure (from trace analysis):
  - bf16 LoadStationary streams 2 rows/cycle: a 128x128 W tile costs
    ~27ns, so one step's 16 LDW+MM pairs are only ~432ns of PE work.
  - The serial bottleneck is the alpha turnaround: last matmul complete
    (+167ns) -> sem (+35) -> DVE multiply (+173) -> sem (+34) ~= 410ns
    of PE idle per step in the single-chain baseline.
  - Fix: split the per-core batch (8) into NCHAINS=4 independent
    chains of 2 that advance round-robin; while one chain's alpha is in
    the psum->DVE->SBUF turnaround, the other three keep the PE busy.
    One fused DVE op per chain-step (all 4 psum j-chunks at once) keeps
    the Vector engine at 4x173ns per ~1.7us period.
  - Each chain owns one 2-bank psum tile [P, bank, half, 256] with
    jc = bank*2 + half; exactly one start per bank per step (psum
    pending-zero covers the whole 2KB region), stop on the last matmul
    touching the bank.

Sharding: data-parallel over batch (64 -> 8 per core), W replicated.
"""

import numpy as np
import ml_dtypes

Z = 512
X = 10000
SEQ = 256
B = 64
NCORES = 8
BS = B // NCORES    # 8 batch per core
NCHAINS = 2
CBS = BS // NCHAINS  # 2 batch per chain
P = 128
ZC = Z // P    # 4 z-chunks
SHIFT = 9.2
USE_FP8 = False
FORCE_ORDER = False
SW = 2048.0 if USE_FP8 else 1.0  # W scale (fp8: max entry ~203 < 240)
ASC = 1.5      # initial alpha mean (vector max/mean ~92 -> max ~140 < 240)
TCH = 51       # eobs t-chunk (5 * 51 = 255)
NCH = (SEQ - 1) // TCH

_NC_CACHE = {}


def _build_nc():
    if "nc" in _NC_CACHE:
        return _NC_CACHE["nc"]
    from concourse import bacc
    import concourse.mybir as mybir
    import concourse.tile as tile
    from concourse.tile_rust import add_dep_helper

    bf16 = mybir.dt.bfloat16
    adt = mybir.dt.float8e4 if USE_FP8 else bf16
    f32 = mybir.dt.float32

    nc = bacc.Bacc("TRN2", target_bir_lowering=False, debug=False,
                   num_devices=NCORES)

    # w[p, ic, j] = Wq[ic*128 + p, j]
    w_d = nc.dram_tensor("w", [P, ZC, Z], adt, kind="ExternalInput")
    # eobs[p, t, bank, half, b] = eobs_scaled[t, (bank*2+half)*128+p, b]
    eobs_d = nc.dram_tensor("eobs", [P, SEQ - 1, 2, 2, BS], bf16,
                            kind="ExternalInput")
    ae0_d = nc.dram_tensor("ae0", [P, 2, 2, BS], adt, kind="ExternalInput")
    out_d = nc.dram_tensor("out", [1, BS], f32, kind="ExternalOutput")

    with tile.TileContext(nc) as tc:
        with (
            tc.tile_pool(name="constp", bufs=1) as constp,
            tc.tile_pool(name="aep", bufs=2) as aep,
            tc.tile_pool(name="psp", bufs=1, space="PSUM") as psp,
            tc.tile_pool(name="finp", bufs=1) as finp,
        ):
            w_sb = constp.tile([P, ZC, Z], adt, name="w_sb")
            nc.sync.dma_start(out=w_sb[:], in_=w_d[:])

            ae_init = constp.tile([P, 2, 2, BS], adt, name="ae_init")
            nc.sync.dma_start(out=ae_init[:], in_=ae0_d[:])

            ones_sb = constp.tile([P, 1], adt, name="ones_sb")
            nc.vector.memset(ones_sb[:], 1.0)
            # Load the Ln table set early so the final log doesn't stall.
            scr_in = finp.tile([P, 1], f32, name="scr_in")
            nc.vector.memset(scr_in[:], 1.0)
            scratch = finp.tile([P, 1], f32, name="scratch")
            nc.scalar.activation(scratch[:], scr_in[:],
                                 mybir.ActivationFunctionType.Ln)

            eobs_sb = []
            for k in range(NCH):
                et = constp.tile([P, TCH, 2, 2, BS], bf16, name=f"eobs_{k}",
                                 tag=f"eobs_{k}")
                nc.sync.dma_start(out=et[:],
                                  in_=eobs_d[:, k * TCH:(k + 1) * TCH])
                eobs_sb.append(et)

            pst = [psp.tile([P, 2, 2, 256], f32, name=f"ps_c{c}",
                            tag=f"ps_c{c}") for c in range(NCHAINS)]

            # prev[c][ic] = alpha chunk [P, CBS] for rows ic*128..+127
            prev = [[ae_init[:, ic // 2, ic % 2,
                             c * CBS:(c + 1) * CBS] for ic in range(ZC)]
                    for c in range(NCHAINS)]
            prev_mm = None
            for t in range(1, SEQ):
                k, toff = divmod(t - 1, TCH)
                for c in range(NCHAINS):
                    ps = pst[c]
                    for ic in range(ZC):
                        for jc in range(ZC):
                            m = nc.tensor.matmul(
                                ps[:, jc // 2, jc % 2, 0:CBS],
                                w_sb[:, ic, jc * P:(jc + 1) * P],
                                prev[c][ic],
                                start=(ic == 0 and jc % 2 == 0),
                                stop=(ic == ZC - 1 and jc % 2 == 1),
                                skip_group_check=True,
                            )
                            if FORCE_ORDER and prev_mm is not None:
                                add_dep_helper(m.ins, prev_mm, sync=False,
                                               reason="mm-order")
                            prev_mm = m.ins
                    ae = aep.tile([P, 2, 2, CBS], adt, tag=f"ae_c{c}",
                                  name=f"ae_c{c}_{t}")
                    nc.vector.tensor_mul(
                        ae[:], ps[:, :, :, 0:CBS],
                        eobs_sb[k][:, toff, :, :, c * CBS:(c + 1) * CBS])
                    prev[c] = [ae[:, ic // 2, ic % 2, :] for ic in range(ZC)]

            # Final: s[b] = sum_z aD_255[z, b] via ones-matmuls.
            lg = finp.tile([1, BS], f32, name="lg")
            for c in range(NCHAINS):
                psf = psp.tile([1, CBS], f32, tag=f"ps_c{c}",
                               name=f"ps_fin{c}")
                for ic in range(ZC):
                    nc.tensor.matmul(psf[:], ones_sb[:], prev[c][ic],
                                     start=(ic == 0), stop=(ic == ZC - 1))
                nc.scalar.activation(lg[:, c * CBS:(c + 1) * CBS], psf[:],
                                     mybir.ActivationFunctionType.Ln)
            nc.sync.dma_start(out=out_d[:], in_=lg[:])

    nc.compile()
    _NC_CACHE["nc"] = nc
    return nc


def _log_softmax64(x, axis):
    x = np.asarray(x, np.float64)
    m = x.max(axis=axis, keepdims=True)
    return x - m - np.log(np.exp(x - m).sum(axis=axis, keepdims=True))


def host_prep(input_ids, T, pi, emit):
    """Numpy prep: normalize params, gather per-step emissions, shard."""
    ids = np.asarray(input_ids).astype(np.int64)
    T_log = _log_softmax64(T, 0)
    pi_log = _log_softmax64(pi, 0)
    emit_log = _log_softmax64(emit, 0)
    W = np.exp(T_log).T  # [i, j] = p(j|i)
    obs = emit_log[ids]  # [256, 64, 512]
    eobs = np.exp(obs[1:] + SHIFT)  # [255, 64, 512]
    ae0 = np.exp(obs[0] + pi_log[None, :])  # [64, 512]

    # rank-1 growth predictor: stationary distribution of W^T
    v = np.ones(Z) / Z
    M = W.T
    for _ in range(50):
        v = M @ v
        v /= v.sum()
    sigma = np.einsum('j,tbj->tb', v, eobs)  # [255, 64]

    adt = ml_dtypes.float8_e4m3 if USE_FP8 else ml_dtypes.bfloat16
    bf = ml_dtypes.bfloat16
    # w_pack[p, ic, j] = W[ic*128 + p, j] * SW
    w_pack = np.ascontiguousarray(
        (W * SW).reshape(ZC, P, Z).transpose(1, 0, 2).astype(adt))

    a0mean = ae0.mean(axis=1)  # [64]
    a0 = (ae0 / a0mean[:, None] * ASC)  # [64, 512]
    logC = np.log(a0mean) - np.log(ASC) + np.log(sigma).sum(axis=0)  # [64]

    eobs_s = eobs / (sigma[:, :, None] * SW)  # [255, 64, 512]

    in_maps = []
    for c in range(NCORES):
        bsl = slice(c * BS, (c + 1) * BS)
        e = eobs_s[:, bsl, :].reshape(SEQ - 1, BS, 2, 2, P)
        e = np.ascontiguousarray(e.transpose(4, 0, 2, 3, 1).astype(bf))
        a = a0[bsl, :].reshape(BS, 2, 2, P)
        a = np.ascontiguousarray(a.transpose(3, 1, 2, 0).astype(adt))
        in_maps.append({"w": w_pack, "eobs": e, "ae0": a})
    return in_maps, logC


def kernel(input_ids, T, pi, emit, _trace=False):
    from concourse.bass_utils import run_bass_kernel_spmd

    nc = _build_nc()
    in_maps, logC = host_prep(input_ids, T, pi, emit)
    r = run_bass_kernel_spmd(nc, in_maps, core_ids=list(range(NCORES)),
                             trace=_trace)
    lg = np.concatenate([r.results[c]["out"][0] for c in range(NCORES)])
    if _trace:
        kernel.last_results = r
    out = -(lg.astype(np.float64) + logC - (SEQ - 1) * SHIFT)
    return out.astype(np.float32)


# revision 19
# speedup vs baseline: 1.0080x; 1.0080x over previous
"""HMM forward (negative log-marginal) on 8 TRN2 NeuronCores.

Algorithm: the log-space recurrence
    alpha_t[b,j] = obs_t[b,j] + LSE_i(alpha_{t-1}[b,i] + T_log[j,i])
is run in linear space with per-step host-precomputed normalizers:
    aD_t[j,b] = (eobs_t[j,b] / (sigma_tb * SW)) * sum_i Wq[i,j] * aD_{t-1}[i,b]
where sigma_tb = pi_star . eobs_t[:,b] is a rank-1 prediction of the
per-step growth that keeps aD ~ O(1), and the log scales are summed on
the host:
    -log p = -(log sum_j aD_255 + sum_t log sigma_tb + logC0 - 255*SHIFT).

Performance structure (from trace analysis):
  - bf16 LoadStationary streams 2 rows/cycle: a 128x128 W tile costs
    ~27ns, so one step's 16 LDW+MM pairs are only ~432ns of PE work.
  - The serial bottleneck is the alpha turnaround: last matmul complete
    (+167ns) -> sem (+35) -> DVE multiply (+173) -> sem (+34) ~= 410ns
    of PE idle per step in the single-chain baseline.
  - Fix: split the per-core batch (8) into NCHAINS=4 independent
    chains of 2 that advance round-robin; while one chain's alpha is in
    the psum->DVE->SBUF turnaround, the other three keep the PE busy.
    One fused DVE op per chain-step (all 4 psum j-chunks at once) keeps
    the Vector engine at 4x173ns per ~1.7us period.
  - Each chain owns one 2-bank psum tile [P, bank, half, 256] with
    jc = bank*2 + half; exactly one start per bank per step (psum
    pending-zero covers the whole 2KB region), stop on the last matmul
    touching the bank.

Sharding: data-parallel over batch (64 -> 8 per core), W replicated.
"""

import numpy as np
import ml_dtypes

Z = 512
X = 10000
SEQ = 256
B = 64
NCORES = 8
BS = B // NCORES    # 8 batch per core
NCHAINS = 2
CBS = BS // NCHAINS  # 2 batch per chain
P = 128
ZC = Z // P    # 4 z-chunks
SHIFT = 9.2
USE_FP8 = True
FORCE_ORDER = False
SW = 2048.0 if USE_FP8 else 1.0  # W scale (fp8: max entry ~203 < 240)
ASC = 1.5      # initial alpha mean (vector max/mean ~92 -> max ~140 < 240)
TCH = 51       # eobs t-chunk (5 * 51 = 255)
NCH = (SEQ - 1) // TCH

_NC_CACHE = {}


def _build_nc():
    if "nc" in _NC_CACHE:
        return _NC_CACHE["nc"]
    from concourse import bacc
    import concourse.mybir as mybir
    import concourse.tile as tile
    from concourse.tile_rust import add_dep_helper

    bf16 = mybir.dt.bfloat16
    adt = mybir.dt.float8e4 if USE_FP8 else bf16
    f32 = mybir.dt.float32

    nc = bacc.Bacc("TRN2", target_bir_lowering=False, debug=False,
                   num_devices=NCORES)

    # w[p, ic, j] = Wq[ic*128 + p, j]
    w_d = nc.dram_tensor("w", [P, ZC, Z], adt, kind="ExternalInput")
    # eobs[p, t, bank, half, b] = eobs_scaled[t, (bank*2+half)*128+p, b]
    eobs_d = nc.dram_tensor("eobs", [P, SEQ - 1, 2, 2, BS], bf16,
                            kind="ExternalInput")
    ae0_d = nc.dram_tensor("ae0", [P, 2, 2, BS], adt, kind="ExternalInput")
    out_d = nc.dram_tensor("out", [1, BS], f32, kind="ExternalOutput")

    with tile.TileContext(nc) as tc:
        with (
            tc.tile_pool(name="constp", bufs=1) as constp,
            tc.tile_pool(name="aep", bufs=2) as aep,
            tc.tile_pool(name="psp", bufs=1, space="PSUM") as psp,
            tc.tile_pool(name="finp", bufs=1) as finp,
        ):
            w_sb = constp.tile([P, ZC, Z], adt, name="w_sb")
            nc.sync.dma_start(out=w_sb[:], in_=w_d[:])

            ae_init = constp.tile([P, 2, 2, BS], adt, name="ae_init")
            nc.sync.dma_start(out=ae_init[:], in_=ae0_d[:])

            ones_sb = constp.tile([P, 1], adt, name="ones_sb")
            nc.vector.memset(ones_sb[:], 1.0)
            # Load the Ln table set early so the final log doesn't stall.
            scr_in = finp.tile([P, 1], f32, name="scr_in")
            nc.vector.memset(scr_in[:], 1.0)
            scratch = finp.tile([P, 1], f32, name="scratch")
            nc.scalar.activation(scratch[:], scr_in[:],
                                 mybir.ActivationFunctionType.Ln)

            eobs_sb = []
            for k in range(NCH):
                et = constp.tile([P, TCH, 2, 2, BS], bf16, name=f"eobs_{k}",
                                 tag=f"eobs_{k}")
                nc.sync.dma_start(out=et[:],
                                  in_=eobs_d[:, k * TCH:(k + 1) * TCH])
                eobs_sb.append(et)

            pst = [psp.tile([P, 2, 2, 256], f32, name=f"ps_c{c}",
                            tag=f"ps_c{c}") for c in range(NCHAINS)]

            # prev[c][ic] = alpha chunk [P, CBS] for rows ic*128..+127
            prev = [[ae_init[:, ic // 2, ic % 2,
                             c * CBS:(c + 1) * CBS] for ic in range(ZC)]
                    for c in range(NCHAINS)]
            prev_mm = None
            for t in range(1, SEQ):
                k, toff = divmod(t - 1, TCH)
                for c in range(NCHAINS):
                    ps = pst[c]
                    for ic in range(ZC):
                        for jc in range(ZC):
                            m = nc.tensor.matmul(
                                ps[:, jc // 2, jc % 2, 0:CBS],
                                w_sb[:, ic, jc * P:(jc + 1) * P],
                                prev[c][ic],
                                start=(ic == 0 and jc % 2 == 0),
                                stop=(ic == ZC - 1 and jc % 2 == 1),
                                skip_group_check=True,
                            )
                            if FORCE_ORDER and prev_mm is not None:
                                add_dep_helper(m.ins, prev_mm, sync=False,
                                               reason="mm-order")
                            prev_mm = m.ins
                    ae = aep.tile([P, 2, 2, CBS], adt, tag=f"ae_c{c}",
                                  name=f"ae_c{c}_{t}")
                    nc.vector.tensor_mul(
                        ae[:], ps[:, :, :, 0:CBS],
                        eobs_sb[k][:, toff, :, :, c * CBS:(c + 1) * CBS])
                    prev[c] = [ae[:, ic // 2, ic % 2, :] for ic in range(ZC)]

            # Final: s[b] = sum_z aD_255[z, b] via ones-matmuls.
            lg = finp.tile([1, BS], f32, name="lg")
            for c in range(NCHAINS):
                psf = psp.tile([1, CBS], f32, tag=f"ps_c{c}",
                               name=f"ps_fin{c}")
                for ic in range(ZC):
                    nc.tensor.matmul(psf[:], ones_sb[:], prev[c][ic],
                                     start=(ic == 0), stop=(ic == ZC - 1))
                nc.scalar.activation(lg[:, c * CBS:(c + 1) * CBS], psf[:],
                                     mybir.ActivationFunctionType.Ln)
            nc.sync.dma_start(out=out_d[:], in_=lg[:])

    nc.compile()
    _NC_CACHE["nc"] = nc
    return nc


def _log_softmax64(x, axis):
    x = np.asarray(x, np.float64)
    m = x.max(axis=axis, keepdims=True)
    return x - m - np.log(np.exp(x - m).sum(axis=axis, keepdims=True))


def host_prep(input_ids, T, pi, emit):
    """Numpy prep: normalize params, gather per-step emissions, shard."""
    ids = np.asarray(input_ids).astype(np.int64)
    T_log = _log_softmax64(T, 0)
    pi_log = _log_softmax64(pi, 0)
    emit_log = _log_softmax64(emit, 0)
    W = np.exp(T_log).T  # [i, j] = p(j|i)
    obs = emit_log[ids]  # [256, 64, 512]
    eobs = np.exp(obs[1:] + SHIFT)  # [255, 64, 512]
    ae0 = np.exp(obs[0] + pi_log[None, :])  # [64, 512]

    # rank-1 growth predictor: stationary distribution of W^T
    v = np.ones(Z) / Z
    M = W.T
    for _ in range(50):
        v = M @ v
        v /= v.sum()
    sigma = np.einsum('j,tbj->tb', v, eobs)  # [255, 64]

    adt = ml_dtypes.float8_e4m3 if USE_FP8 else ml_dtypes.bfloat16
    bf = ml_dtypes.bfloat16
    # w_pack[p, ic, j] = W[ic*128 + p, j] * SW
    w_pack = np.ascontiguousarray(
        (W * SW).reshape(ZC, P, Z).transpose(1, 0, 2).astype(adt))

    a0mean = ae0.mean(axis=1)  # [64]
    a0 = (ae0 / a0mean[:, None] * ASC)  # [64, 512]
    logC = np.log(a0mean) - np.log(ASC) + np.log(sigma).sum(axis=0)  # [64]

    eobs_s = eobs / (sigma[:, :, None] * SW)  # [255, 64, 512]

    in_maps = []
    for c in range(NCORES):
        bsl = slice(c * BS, (c + 1) * BS)
        e = eobs_s[:, bsl, :].reshape(SEQ - 1, BS, 2, 2, P)
        e = np.ascontiguousarray(e.transpose(4, 0, 2, 3, 1).astype(bf))
        a = a0[bsl, :].reshape(BS, 2, 2, P)
        a = np.ascontiguousarray(a.transpose(3, 1, 2, 0).astype(adt))
        in_maps.append({"w": w_pack, "eobs": e, "ae0": a})
    return in_maps, logC


def kernel(input_ids, T, pi, emit, _trace=False):
    from concourse.bass_utils import run_bass_kernel_spmd

    nc = _build_nc()
    in_maps, logC = host_prep(input_ids, T, pi, emit)
    r = run_bass_kernel_spmd(nc, in_maps, core_ids=list(range(NCORES)),
                             trace=_trace)
    lg = np.concatenate([r.results[c]["out"][0] for c in range(NCORES)])
    if _trace:
        kernel.last_results = r
    out = -(lg.astype(np.float64) + logC - (SEQ - 1) * SHIFT)
    return out.astype(np.float32)
